# revision 21
# baseline (speedup 1.0000x reference)
"""Trainium2 Bass kernel for nn_BeliefTransformerBlock.

Reference model (per sample b):
    xn   = layer_norm(x) * ln1_w
    q,k,v = split(xn @ qkv_w.T); heads of 128
    att  = softmax(q k^T / sqrt(128) + key_mask * -1e10)
    x2   = att @ v + x
    x2n  = layer_norm(x2) * ln2_w
    out  = gelu(x2n @ fc_w.T, exact) @ proj_w.T + x2n
    returns (out, att)

Sharding: data-parallel, one sample per NeuronCore (B=8, 8 cores).

On-chip strategy (per core; contraction dims on partitions; host-side
transposes/folds are free):
  - x^T [d, s] streamed in; LN1 stats via ones-matmul partition reductions,
    broadcast with gpsimd partition_broadcast, x~^T on DVE.
  - QKV as [e, s] (weights stationary), v as [s, e].
  - scores^T [kp, qp] = k~_h^T . q~_h^T  (transpose-free chain)
  - E = exp(scores^T + mask_bias[kp])  (ragged key mask folded into the
    per-partition exp bias -> masking costs nothing, one program fits all)
  - Z via ones-matmul over kp ; y^T_h = matmul(lhsT=v, rhs=E)
  - att = E/Z (broadcast recip), out written transposed; host untransposes.
Matmuls run in float32r (fp32, 11-bit explicit mantissa, full PE rate).
"""
import sys
import types

sys.path.insert(0, "/opt/trn_rl_repo")

import numpy as np
from contextlib import ExitStack

import concourse.bass as bass  # noqa: F401
import concourse.tile as tile
from concourse import bacc, mybir
from concourse.bass_utils import run_bass_kernel_spmd

B, S, D, H = 8, 1024, 1024, 8
HD = D // H
NEG = -1e10
EPS = 1e-5
P = 128
NT = D // P          # d tiles
F32 = mybir.dt.float32
F32R = mybir.dt.float32r
AF = mybir.ActivationFunctionType
OP = mybir.AluOpType


def round_f32r(x: np.ndarray) -> np.ndarray:
    """Round f32 to f32r (11 explicit mantissa bits, round-to-nearest-even)."""
    b = np.ascontiguousarray(x, dtype=np.float32).view(np.uint32)
    lo = b & np.uint32(0xFFF)
    hi = b & np.uint32(0xFFFFF000)
    round_up = (lo > 0x800) | ((lo == 0x800) & (((hi >> 12) & 1) == 1))
    out = hi + (round_up.astype(np.uint32) << 12)
    return out.view(np.float32)


def _chunks(n):
    cw = min(512, n)
    return [(i * cw, cw) for i in range(n // cw)]


def build(sp=S):
    """Build the single-core program (same program on all 8 cores; the
    ragged mask enters as data). sp = sequence length (small configs can
    run under CoreSim)."""
    nts = sp // P
    sch = _chunks(sp)       # s-dim chunks (moving dim)
    ech = _chunks(D)        # e-dim chunks for v
    isq2 = float(1.0 / np.sqrt(2.0))

    nc = bacc.Bacc("TRN2", target_bir_lowering=False, debug=False, num_devices=8)

    # ---- DRAM tensors (per-core) ----
    xr_d = nc.dram_tensor("xr", [NT, P, sp], F32R, kind="ExternalInput").ap()
    xf_d = nc.dram_tensor("xf", [NT, P, sp], F32, kind="ExternalInput").ap()
    wqk_d = nc.dram_tensor("wqk", [H, NT, P, 2 * P], F32R, kind="ExternalInput").ap()
    wv_d = nc.dram_tensor("wv", [NT, P, D], F32R, kind="ExternalInput").ap()
    wfc_d = nc.dram_tensor("wfc", [NT, P, D], F32R, kind="ExternalInput").ap()
    wpj_d = nc.dram_tensor("wpj", [NT, P, D], F32R, kind="ExternalInput").ap()
    mask_d = nc.dram_tensor("maskT", [P, nts], F32, kind="ExternalInput").ap()

    att_d = nc.dram_tensor("attT", [H, sp, sp], F32R, kind="ExternalOutput").ap()
    out_d = nc.dram_tensor("outT", [NT, P, sp], F32, kind="ExternalOutput").ap()

    with tile.TileContext(nc) as tc, ExitStack() as ctx:
        consts = ctx.enter_context(tc.tile_pool(name="consts", bufs=1))
        dram = ctx.enter_context(tc.tile_pool(name="dram", bufs=1, space="DRAM"))

        ones_f = consts.tile([P, 1], F32)
        nc.any.memset(ones_f[:], 1.0)
        ones = consts.tile([P, 1], F32R)
        nc.vector.tensor_copy(ones[:], ones_f[:])
        maskT = consts.tile([P, nts], F32)
        nc.sync.dma_start(maskT[:], mask_d)
        eps_c = consts.tile([1, 1], F32)
        nc.any.memset(eps_c[:], EPS)

        x2_dram = dram.tile([NT, P, sp], F32, name="x2_dram")

        with ExitStack() as sA:
            xT_pool = sA.enter_context(tc.tile_pool(name="xT", bufs=NT))
            vsb_pool = sA.enter_context(tc.tile_pool(name="vsb", bufs=nts))
            ps_mm = sA.enter_context(tc.tile_pool(name="ps_mm", bufs=2, space="PSUM"))

            xT = [xT_pool.tile([P, sp], F32R, tag="xT", name=f"xT_{i}") for i in range(NT)]
            v_sb = [vsb_pool.tile([P, D], F32R, tag="vsb", name=f"vsb_{i}") for i in range(nts)]

            # ---------- phase 0: LN1 stats + x~^T ; phase v ----------
            with ExitStack() as s0:
                stream = s0.enter_context(tc.tile_pool(name="stream", bufs=3))
                sq_pool = s0.enter_context(tc.tile_pool(name="sq", bufs=2))
                stat_pool = s0.enter_context(tc.tile_pool(name="stat", bufs=1))
                bc_pool = s0.enter_context(tc.tile_pool(name="bc", bufs=1))
                wv_pool = s0.enter_context(tc.tile_pool(name="wv", bufs=NT))
                ps_stat = s0.enter_context(
                    tc.tile_pool(name="ps_stat", bufs=2, space="PSUM"))

                ps_mu = [ps_stat.tile([1, cw], F32, tag="ps_mu", name=f"ps_mu_{i}") for i, (_, cw) in enumerate(sch)]
                ps_sq = [ps_stat.tile([1, cw], F32, tag="ps_sq", name=f"ps_sq_{i}") for i, (_, cw) in enumerate(sch)]
                for dt in range(NT):
                    xr_t = stream.tile([P, sp], F32R, tag="xr")
                    nc.sync.dma_start(xr_t[:], xr_d[dt])
                    xsq = sq_pool.tile([P, sp], F32R, tag="xsq")
                    nc.vector.tensor_mul(xsq[:], xr_t[:].bitcast(F32),
                                         xr_t[:].bitcast(F32))
                    for ci, (c0, cw) in enumerate(sch):
                        nc.tensor.matmul(ps_mu[ci][:], ones[:],
                                         xr_t[:, c0:c0 + cw],
                                         start=(dt == 0), stop=(dt == NT - 1))
                    for ci, (c0, cw) in enumerate(sch):
                        nc.tensor.matmul(ps_sq[ci][:], ones[:],
                                         xsq[:, c0:c0 + cw],
                                         start=(dt == 0), stop=(dt == NT - 1))

                mu = stat_pool.tile([1, sp], F32, tag="mu")
                msq = stat_pool.tile([1, sp], F32, tag="msq")
                for ci, (c0, cw) in enumerate(sch):
                    nc.scalar.mul(mu[:, c0:c0 + cw], ps_mu[ci][:], 1.0 / D)
                    nc.scalar.mul(msq[:, c0:c0 + cw], ps_sq[ci][:], 1.0 / D)
                var = stat_pool.tile([1, sp], F32, tag="var")
                nc.vector.tensor_mul(var[:], mu[:], mu[:])
                nc.vector.tensor_sub(var[:], msq[:], var[:])
                std = stat_pool.tile([1, sp], F32, tag="std")
                nc.scalar.activation(std[:], var[:], AF.Sqrt, bias=eps_c[:])
                rstd = stat_pool.tile([1, sp], F32, tag="rstd")
                nc.vector.reciprocal_approx_fast(rstd[:], std[:])

                Rmu = bc_pool.tile([P, sp], F32, tag="Rmu")
                nc.gpsimd.partition_broadcast(Rmu[:], mu[:])
                Rrstd = bc_pool.tile([P, sp], F32, tag="Rrstd")
                nc.gpsimd.partition_broadcast(Rrstd[:], rstd[:])

                for dt in range(NT):
                    xf_t = stream.tile([P, sp], F32, tag="xf")
                    nc.sync.dma_start(xf_t[:], xf_d[dt])
                    xc = stream.tile([P, sp], F32, tag="xc")
                    nc.vector.tensor_sub(xc[:], xf_t[:], Rmu[:])
                    nc.vector.tensor_mul(xT[dt][:], xc[:], Rrstd[:])

                # v = x~ @ Wv^T  (v in [s, e] layout)
                wv = []
                for dt in range(NT):
                    t = wv_pool.tile([P, D], F32R, tag="wv")
                    nc.scalar.dma_start(t[:], wv_d[dt])
                    wv.append(t)
                for st in range(nts):
                    for (c0, cw) in ech:
                        ps = ps_mm.tile([P, 512], F32, tag="ps_mm", name="ps_v")[:, :cw]
                        for dt in range(NT):
                            nc.tensor.matmul(ps, xT[dt][:, st * P:(st + 1) * P],
                                             wv[dt][:, c0:c0 + cw],
                                             start=(dt == 0), stop=(dt == NT - 1))
                        nc.scalar.copy(v_sb[st][:, c0:c0 + cw], ps)

            # ---------- heads ----------
            with ExitStack() as sH:
                wqk_pool = sH.enter_context(tc.tile_pool(name="wqk", bufs=3))
                qk_pool = sH.enter_context(tc.tile_pool(name="qk", bufs=3))
                E_pool = sH.enter_context(
                    tc.tile_pool(name="E", bufs=max(8, 4 * nts - 4)))
                ysb_pool = sH.enter_context(tc.tile_pool(name="ysb", bufs=2))
                x2h_pool = sH.enter_context(tc.tile_pool(name="x2h", bufs=2))
                rz_pool = sH.enter_context(tc.tile_pool(name="rz", bufs=1))
                rzb_pool = sH.enter_context(tc.tile_pool(name="rzb", bufs=2))
                xf2_pool = sH.enter_context(tc.tile_pool(name="xf2", bufs=2))
                ps_s_pool = sH.enter_context(
                    tc.tile_pool(name="ps_s", bufs=2, space="PSUM"))
                ps_z_pool = sH.enter_context(
                    tc.tile_pool(name="ps_z", bufs=2, space="PSUM"))
                ps_y_pool = sH.enter_context(
                    tc.tile_pool(name="ps_y", bufs=2, space="PSUM"))

                wqk_tiles = {}

                def _load_wqk(hh):
                    t = wqk_pool.tile([P, NT, 2 * P], F32R, tag="wqk",
                                      name=f"wqk_{hh}")
                    nc.sync.dma_start(t[:], wqk_d[hh].rearrange("t p e -> p t e"))
                    wqk_tiles[hh] = t

                _load_wqk(0)
                if H > 1:
                    _load_wqk(1)
                for h in range(H):
                    if h + 2 < H:
                        _load_wqk(h + 2)
                    xf2 = xf2_pool.tile([P, sp], F32, tag="xf2")
                    nc.sync.dma_start(xf2[:], xf_d[h])
                    wqk_h = wqk_tiles.pop(h)

                    q_sb = qk_pool.tile([P, sp], F32R, tag="qk")
                    k_sb = qk_pool.tile([P, sp], F32R, tag="qk")
                    for (c0, cw) in sch:
                        ps = ps_mm.tile([P, 512], F32, tag="ps_mm", name="ps_q")[:, :cw]
                        for dt in range(NT):
                            nc.tensor.matmul(ps, wqk_h[:, dt, 0:P],
                                             xT[dt][:, c0:c0 + cw],
                                             start=(dt == 0), stop=(dt == NT - 1))
                        nc.scalar.copy(q_sb[:, c0:c0 + cw], ps)
                        ps2 = ps_mm.tile([P, 512], F32, tag="ps_mm", name="ps_k")[:, :cw]
                        for dt in range(NT):
                            nc.tensor.matmul(ps2, wqk_h[:, dt, P:2 * P],
                                             xT[dt][:, c0:c0 + cw],
                                             start=(dt == 0), stop=(dt == NT - 1))
                        nc.scalar.copy(k_sb[:, c0:c0 + cw], ps2)

                    ps_z = [ps_z_pool.tile([1, cw], F32, tag="ps_z", name=f"ps_z_{i}")
                            for i, (_, cw) in enumerate(sch)]
                    ps_y = [ps_y_pool.tile([P, cw], F32, tag="ps_y", name=f"ps_y_{i}")
                            for i, (_, cw) in enumerate(sch)]
                    E = {}
                    for kt in range(nts):
                        for ci, (c0, cw) in enumerate(sch):
                            Et = E_pool.tile([P, 512], F32R, tag="E", name="Et")[:, :cw]
                            ps = ps_s_pool.tile([P, 512], F32, tag="ps_s", name="ps_sc")[:, :cw]
                            nc.tensor.matmul(ps, k_sb[:, kt * P:(kt + 1) * P],
                                             q_sb[:, c0:c0 + cw],
                                             start=True, stop=True)
                            nc.scalar.activation(Et, ps, AF.Exp,
                                                 bias=maskT[:, kt:kt + 1])
                            nc.tensor.matmul(ps_z[ci][:], ones[:], Et,
                                             start=(kt == 0), stop=(kt == nts - 1))
                            nc.tensor.matmul(ps_y[ci][:],
                                             v_sb[kt][:, h * P:(h + 1) * P], Et,
                                             start=(kt == 0), stop=(kt == nts - 1))
                            E[(kt, ci)] = Et

                    z = rz_pool.tile([1, sp], F32, tag="z")
                    for ci, (c0, cw) in enumerate(sch):
                        nc.scalar.copy(z[:, c0:c0 + cw], ps_z[ci][:])
                    rz = rz_pool.tile([1, sp], F32, tag="rz")
                    nc.vector.reciprocal_approx_fast(rz[:], z[:])
                    Rz = rzb_pool.tile([P, sp], F32, tag="Rz")
                    nc.gpsimd.partition_broadcast(Rz[:], rz[:])

                    # free ps_y early: stage y into SBUF via ACT
                    y_sb = ysb_pool.tile([P, sp], F32, tag="y_sb")
                    for ci, (c0, cw) in enumerate(sch):
                        nc.scalar.copy(y_sb[:, c0:c0 + cw], ps_y[ci][:])

                    # x2^T[h] = y^T_h / Z + x[h]  (before the att flood so its
                    # store lands early on the sync queue)
                    x2h = x2h_pool.tile([P, sp], F32, tag="x2h")
                    for ci, (c0, cw) in enumerate(sch):
                        nc.vector.tensor_mul(x2h[:, c0:c0 + cw],
                                             y_sb[:, c0:c0 + cw],
                                             Rz[:, c0:c0 + cw])
                        nc.vector.tensor_add(x2h[:, c0:c0 + cw],
                                             x2h[:, c0:c0 + cw],
                                             xf2[:, c0:c0 + cw])
                    nc.sync.dma_start(x2_dram[h], x2h[:])

                    # att rows: normalize E in place (DVE/GPSIMD split) + stream
                    for kt in range(nts):
                        for ci, (c0, cw) in enumerate(sch):
                            Et = E[(kt, ci)]
                            eng = (nc.gpsimd if (h == H - 1 or kt % 3 == 2)
                                   else nc.vector)
                            eng.tensor_mul(Et, Et.bitcast(F32), Rz[:, c0:c0 + cw])
                            nc.sync.dma_start(
                                att_d[h, kt * P:(kt + 1) * P, c0:c0 + cw], Et)

        # ---------- phase 3: LN2 + FC + proj ----------
        with ExitStack() as sB:
            x2f_pool = sB.enter_context(tc.tile_pool(name="x2f", bufs=NT))
            sq2_pool = sB.enter_context(tc.tile_pool(name="sq2", bufs=2))
            stat2_pool = sB.enter_context(tc.tile_pool(name="stat2", bufs=1))
            bc2_pool = sB.enter_context(tc.tile_pool(name="bc2", bufs=1))
            xt2_pool = sB.enter_context(tc.tile_pool(name="xt2", bufs=NT))
            w2_pool = sB.enter_context(tc.tile_pool(name="w2", bufs=NT + 1))
            hT_pool = sB.enter_context(tc.tile_pool(name="hT", bufs=NT))
            tmp_pool = sB.enter_context(tc.tile_pool(name="tmp", bufs=3))
            ps_mm2 = sB.enter_context(
                tc.tile_pool(name="ps_mm2", bufs=4, space="PSUM"))
            ps_stat2 = sB.enter_context(
                tc.tile_pool(name="ps_stat2", bufs=2, space="PSUM"))

            ps_mu2 = [ps_stat2.tile([1, cw], F32, tag="ps_mu2", name=f"ps_mu2_{i}")
                      for i, (_, cw) in enumerate(sch)]
            ps_sq2 = [ps_stat2.tile([1, cw], F32, tag="ps_sq2", name=f"ps_sq2_{i}")
                      for i, (_, cw) in enumerate(sch)]
            x2fs = []
            for dt in range(NT):
                x2f = x2f_pool.tile([P, sp], F32, tag="x2f", name=f"x2f_{dt}")
                nc.scalar.dma_start(x2f[:], x2_dram[dt])
                x2fs.append(x2f)
                r = sq2_pool.tile([P, sp], F32R, tag="x2r")
                nc.scalar.copy(r[:], x2f[:])
                q = sq2_pool.tile([P, sp], F32R, tag="x2sq")
                nc.scalar.activation(q[:], x2f[:], AF.Square)
                for ci, (c0, cw) in enumerate(sch):
                    nc.tensor.matmul(ps_mu2[ci][:], ones[:], r[:, c0:c0 + cw],
                                     start=(dt == 0), stop=(dt == NT - 1))
                for ci, (c0, cw) in enumerate(sch):
                    nc.tensor.matmul(ps_sq2[ci][:], ones[:], q[:, c0:c0 + cw],
                                     start=(dt == 0), stop=(dt == NT - 1))

            mu2 = stat2_pool.tile([1, sp], F32, tag="mu2")
            msq2 = stat2_pool.tile([1, sp], F32, tag="msq2")
            for ci, (c0, cw) in enumerate(sch):
                nc.scalar.mul(mu2[:, c0:c0 + cw], ps_mu2[ci][:], 1.0 / D)
                nc.scalar.mul(msq2[:, c0:c0 + cw], ps_sq2[ci][:], 1.0 / D)
            var2 = stat2_pool.tile([1, sp], F32, tag="var2")
            nc.vector.tensor_mul(var2[:], mu2[:], mu2[:])
            nc.vector.tensor_sub(var2[:], msq2[:], var2[:])
            std2 = stat2_pool.tile([1, sp], F32, tag="std2")
            nc.scalar.activation(std2[:], var2[:], AF.Sqrt, bias=eps_c[:])
            rstd2 = stat2_pool.tile([1, sp], F32, tag="rstd2")
            nc.vector.reciprocal_approx_fast(rstd2[:], std2[:])
            Rmu2 = bc2_pool.tile([P, sp], F32, tag="Rmu2")
            nc.gpsimd.partition_broadcast(Rmu2[:], mu2[:])
            Rrstd2 = bc2_pool.tile([P, sp], F32, tag="Rrstd2")
            nc.gpsimd.partition_broadcast(Rrstd2[:], rstd2[:])

            xt2 = []
            for dt in range(NT):
                t = tmp_pool.tile([P, sp], F32, tag="x2c")
                nc.vector.tensor_sub(t[:], x2fs[dt][:], Rmu2[:])
                u = xt2_pool.tile([P, sp], F32R, tag="xt2")
                nc.vector.tensor_mul(u[:], t[:], Rrstd2[:])
                xt2.append(u)

            # FC + exact gelu: g = h * (0.5 * erf(h/sqrt(2)) + 0.5)
            wfc = []
            for dt in range(NT):
                t = w2_pool.tile([P, D], F32R, tag="w2")
                nc.scalar.dma_start(t[:], wfc_d[dt])
                wfc.append(t)
            hT = [hT_pool.tile([P, sp], F32R, tag="hT", name=f"hT_{i}") for i in range(NT)]
            for et in range(NT):
                for (c0, cw) in sch:
                    ps = ps_mm2.tile([P, 512], F32, tag="ps_mm2", name="ps_fc")[:, :cw]
                    for dt in range(NT):
                        nc.tensor.matmul(ps, wfc[dt][:, et * P:(et + 1) * P],
                                         xt2[dt][:, c0:c0 + cw],
                                         start=(dt == 0), stop=(dt == NT - 1))
                    nc.scalar.activation(hT[et][:, c0:c0 + cw], ps, AF.Gelu)

            wpj = []
            for et in range(NT):
                t = w2_pool.tile([P, D], F32R, tag="w2")
                nc.scalar.dma_start(t[:], wpj_d[et])
                wpj.append(t)
            for dt in range(NT):
                for (c0, cw) in sch:
                    ps = ps_mm2.tile([P, 512], F32, tag="ps_mm2", name="ps_pj")[:, :cw]
                    for et in range(NT):
                        nc.tensor.matmul(ps, wpj[et][:, dt * P:(dt + 1) * P],
                                         hT[et][:, c0:c0 + cw],
                                         start=(et == 0), stop=(et == NT - 1))
                    o = tmp_pool.tile([P, 512], F32, tag="o", name="o")[:, :cw]
                    nc.vector.tensor_add(o, ps, xt2[dt][:, c0:c0 + cw].bitcast(F32))
                    nc.sync.dma_start(out_d[dt, :, c0:c0 + cw], o)

    nc.compile()
    return nc


def prep_core_inputs(x_b, size_b, ln1_w, qkv_w, ln2_w, fc_w, proj_w, sp=S):
    """Host-side prep for one core/sample."""
    nts = sp // P
    xT = np.ascontiguousarray(x_b[:sp].T)                     # [D, sp]
    wq = (qkv_w[0:D] * ln1_w[None, :]) / np.sqrt(HD)          # [e, d]
    wk = qkv_w[D:2 * D] * ln1_w[None, :]
    wv = qkv_w[2 * D:3 * D] * ln1_w[None, :]
    wqkT = np.concatenate([wq, wk], axis=0).T                 # [d, 2D]
    # [H, NT, P, 2P]: head h -> cols [h*128:(h+1)*128] of q and of k
    wqk = np.empty((H, NT, P, 2 * P), np.float32)
    for h in range(H):
        wqk[h, :, :, 0:P] = wqkT[:, h * P:(h + 1) * P].reshape(NT, P, P)
        wqk[h, :, :, P:2 * P] = wqkT[:, D + h * P:D + (h + 1) * P].reshape(NT, P, P)
    wvT = np.ascontiguousarray(wv.T).reshape(NT, P, D)
    wfcT = np.ascontiguousarray((fc_w * ln2_w[None, :]).T).reshape(NT, P, D)
    wpjT = np.ascontiguousarray(proj_w.T).reshape(NT, P, D)

    kp = np.arange(P)[:, None] + P * np.arange(nts)[None, :]  # [P, nts]
    maskT = np.where(kp < int(size_b), 0.0, NEG).astype(np.float32)

    return {
        "xr": round_f32r(xT).reshape(NT, P, sp),
        "xf": np.ascontiguousarray(xT.reshape(NT, P, sp), dtype=np.float32),
        "wqk": round_f32r(wqk),
        "wv": round_f32r(wvT),
        "wfc": round_f32r(wfcT),
        "wpj": round_f32r(wpjT),
        "maskT": maskT,
    }


_NC_CACHE = {}


def _get_nc(sp=S):
    if sp not in _NC_CACHE:
        _NC_CACHE[sp] = build(sp)
    return _NC_CACHE[sp]


def _install_ntff_hook():
    """Shim antenv.axon_hooks so trace=True works under axon (for test.py)."""
    import antenv
    if "antenv.axon_hooks" in sys.modules:
        return
    hooks_mod = types.ModuleType("antenv.axon_hooks")
    _hook = [None]
    hooks_mod.set_axon_ntff_profile_hook = lambda h: _hook.__setitem__(0, h)
    hooks_mod.get_axon_ntff_profile_hook = lambda: _hook[0]
    sys.modules["antenv.axon_hooks"] = hooks_mod
    antenv.axon_hooks = hooks_mod
    try:
        if "/root/.axon_site" not in sys.path:
            sys.path.insert(0, "/root/.axon_site")
        from trn_agent_boot.trn_boot import _ntff_profile_via_ctypes
        h = _ntff_profile_via_ctypes("/opt/axon/libaxon_pjrt.so")
        if h is not None:
            hooks_mod.set_axon_ntff_profile_hook(h)
    except Exception:
        pass


def run(inputs, sp=S, trace=False, **kwargs):
    """Shard, run on 8 cores, gather. Returns (out [B,sp,D], att [B,H,sp,sp], res)."""
    x = np.asarray(inputs["x"], np.float32)
    sizes = np.asarray(inputs["belief_base_sizes"])
    ln1_w = np.asarray(inputs["ln1_w"], np.float32)
    qkv_w = np.asarray(inputs["qkv_w"], np.float32)
    ln2_w = np.asarray(inputs["ln2_w"], np.float32)
    fc_w = np.asarray(inputs["fc_w"], np.float32)
    proj_w = np.asarray(inputs["proj_w"], np.float32)

    nc = _get_nc(sp)
    in_maps = [
        prep_core_inputs(x[b], min(int(sizes[b]), sp), ln1_w, qkv_w, ln2_w,
                         fc_w, proj_w, sp)
        for b in range(B)
    ]
    if trace:
        _install_ntff_hook()
    res = run_bass_kernel_spmd(nc, in_maps, core_ids=list(range(B)),
                               trace=trace, **kwargs)

    out = np.empty((B, sp, D), np.float32)
    att = np.empty((B, H, sp, sp), np.float32)
    for b in range(B):
        r = res.results[b]
        out[b] = r["outT"].reshape(D, sp).T
        att[b] = np.swapaxes(r["attT"], 1, 2)
    return out, att, res


def kernel(**inputs):
    out, att, _ = run(inputs, sp=S, trace=False)
    return out, att


# revision 22
# speedup vs baseline: 1.0039x; 1.0039x over previous
"""Trainium2 Bass kernel for nn_BeliefTransformerBlock.

Reference model (per sample b):
    xn   = layer_norm(x) * ln1_w
    q,k,v = split(xn @ qkv_w.T); heads of 128
    att  = softmax(q k^T / sqrt(128) + key_mask * -1e10)
    x2   = att @ v + x
    x2n  = layer_norm(x2) * ln2_w
    out  = gelu(x2n @ fc_w.T, exact) @ proj_w.T + x2n
    returns (out, att)

Sharding: data-parallel, one sample per NeuronCore (B=8, 8 cores).

On-chip strategy (per core; contraction dims on partitions; host-side
transposes/folds are free):
  - x^T [d, s] streamed in; LN1 stats via ones-matmul partition reductions,
    broadcast with gpsimd partition_broadcast, x~^T on DVE.
  - QKV as [e, s] (weights stationary), v as [s, e].
  - scores^T [kp, qp] = k~_h^T . q~_h^T  (transpose-free chain)
  - E = exp(scores^T + mask_bias[kp])  (ragged key mask folded into the
    per-partition exp bias -> masking costs nothing, one program fits all)
  - Z via ones-matmul over kp ; y^T_h = matmul(lhsT=v, rhs=E)
  - att = E/Z (broadcast recip), out written transposed; host untransposes.
Matmuls run in float32r (fp32, 11-bit explicit mantissa, full PE rate).
"""
import sys
import types

sys.path.insert(0, "/opt/trn_rl_repo")

import numpy as np
from contextlib import ExitStack

import concourse.bass as bass  # noqa: F401
import concourse.tile as tile
from concourse import bacc, mybir
from concourse.bass_utils import run_bass_kernel_spmd

B, S, D, H = 8, 1024, 1024, 8
HD = D // H
NEG = -1e10
EPS = 1e-5
P = 128
NT = D // P          # d tiles
F32 = mybir.dt.float32
F32R = mybir.dt.float32r
AF = mybir.ActivationFunctionType
OP = mybir.AluOpType


def round_f32r(x: np.ndarray) -> np.ndarray:
    """Round f32 to f32r (11 explicit mantissa bits, round-to-nearest-even)."""
    b = np.ascontiguousarray(x, dtype=np.float32).view(np.uint32)
    lo = b & np.uint32(0xFFF)
    hi = b & np.uint32(0xFFFFF000)
    round_up = (lo > 0x800) | ((lo == 0x800) & (((hi >> 12) & 1) == 1))
    out = hi + (round_up.astype(np.uint32) << 12)
    return out.view(np.float32)


def _chunks(n):
    cw = min(512, n)
    return [(i * cw, cw) for i in range(n // cw)]


def build(sp=S):
    """Build the single-core program (same program on all 8 cores; the
    ragged mask enters as data). sp = sequence length (small configs can
    run under CoreSim)."""
    nts = sp // P
    sch = _chunks(sp)       # s-dim chunks (moving dim)
    ech = _chunks(D)        # e-dim chunks for v
    isq2 = float(1.0 / np.sqrt(2.0))

    nc = bacc.Bacc("TRN2", target_bir_lowering=False, debug=False, num_devices=8)

    # ---- DRAM tensors (per-core) ----
    xr_d = nc.dram_tensor("xr", [NT, P, sp], F32R, kind="ExternalInput").ap()
    xf_d = nc.dram_tensor("xf", [NT, P, sp], F32, kind="ExternalInput").ap()
    wqk_d = nc.dram_tensor("wqk", [H, NT, P, 2 * P], F32R, kind="ExternalInput").ap()
    wv_d = nc.dram_tensor("wv", [NT, P, D], F32R, kind="ExternalInput").ap()
    wfc_d = nc.dram_tensor("wfc", [NT, P, D], F32R, kind="ExternalInput").ap()
    wpj_d = nc.dram_tensor("wpj", [NT, P, D], F32R, kind="ExternalInput").ap()
    mask_d = nc.dram_tensor("maskT", [P, nts], F32, kind="ExternalInput").ap()

    att_d = nc.dram_tensor("attT", [H, sp, sp], F32R, kind="ExternalOutput").ap()
    out_d = nc.dram_tensor("outT", [NT, P, sp], F32, kind="ExternalOutput").ap()

    with tile.TileContext(nc) as tc, ExitStack() as ctx:
        consts = ctx.enter_context(tc.tile_pool(name="consts", bufs=1))
        dram = ctx.enter_context(tc.tile_pool(name="dram", bufs=1, space="DRAM"))

        ones_f = consts.tile([P, 1], F32)
        nc.any.memset(ones_f[:], 1.0)
        ones = consts.tile([P, 1], F32R)
        nc.vector.tensor_copy(ones[:], ones_f[:])
        maskT = consts.tile([P, nts], F32)
        nc.sync.dma_start(maskT[:], mask_d)
        eps_c = consts.tile([1, 1], F32)
        nc.any.memset(eps_c[:], EPS)

        x2_dram = dram.tile([NT, P, sp], F32, name="x2_dram")

        with ExitStack() as sA:
            xT_pool = sA.enter_context(tc.tile_pool(name="xT", bufs=NT))
            vsb_pool = sA.enter_context(tc.tile_pool(name="vsb", bufs=nts))
            ps_mm = sA.enter_context(tc.tile_pool(name="ps_mm", bufs=2, space="PSUM"))

            xT = [xT_pool.tile([P, sp], F32R, tag="xT", name=f"xT_{i}") for i in range(NT)]
            v_sb = [vsb_pool.tile([P, D], F32R, tag="vsb", name=f"vsb_{i}") for i in range(nts)]

            # ---------- phase 0: LN1 stats + x~^T ; phase v ----------
            with ExitStack() as s0:
                stream = s0.enter_context(tc.tile_pool(name="stream", bufs=3))
                sq_pool = s0.enter_context(tc.tile_pool(name="sq", bufs=2))
                stat_pool = s0.enter_context(tc.tile_pool(name="stat", bufs=1))
                bc_pool = s0.enter_context(tc.tile_pool(name="bc", bufs=1))
                wv_pool = s0.enter_context(tc.tile_pool(name="wv", bufs=NT))
                ps_stat = s0.enter_context(
                    tc.tile_pool(name="ps_stat", bufs=2, space="PSUM"))

                ps_mu = [ps_stat.tile([1, cw], F32, tag="ps_mu", name=f"ps_mu_{i}") for i, (_, cw) in enumerate(sch)]
                ps_sq = [ps_stat.tile([1, cw], F32, tag="ps_sq", name=f"ps_sq_{i}") for i, (_, cw) in enumerate(sch)]
                for dt in range(NT):
                    xr_t = stream.tile([P, sp], F32R, tag="xr")
                    nc.sync.dma_start(xr_t[:], xr_d[dt])
                    xsq = sq_pool.tile([P, sp], F32R, tag="xsq")
                    nc.vector.tensor_mul(xsq[:], xr_t[:].bitcast(F32),
                                         xr_t[:].bitcast(F32))
                    for ci, (c0, cw) in enumerate(sch):
                        nc.tensor.matmul(ps_mu[ci][:], ones[:],
                                         xr_t[:, c0:c0 + cw],
                                         start=(dt == 0), stop=(dt == NT - 1))
                    for ci, (c0, cw) in enumerate(sch):
                        nc.tensor.matmul(ps_sq[ci][:], ones[:],
                                         xsq[:, c0:c0 + cw],
                                         start=(dt == 0), stop=(dt == NT - 1))

                mu = stat_pool.tile([1, sp], F32, tag="mu")
                msq = stat_pool.tile([1, sp], F32, tag="msq")
                for ci, (c0, cw) in enumerate(sch):
                    nc.scalar.mul(mu[:, c0:c0 + cw], ps_mu[ci][:], 1.0 / D)
                    nc.scalar.mul(msq[:, c0:c0 + cw], ps_sq[ci][:], 1.0 / D)
                var = stat_pool.tile([1, sp], F32, tag="var")
                nc.vector.tensor_mul(var[:], mu[:], mu[:])
                nc.vector.tensor_sub(var[:], msq[:], var[:])
                std = stat_pool.tile([1, sp], F32, tag="std")
                nc.scalar.activation(std[:], var[:], AF.Sqrt, bias=eps_c[:])
                rstd = stat_pool.tile([1, sp], F32, tag="rstd")
                nc.vector.reciprocal_approx_fast(rstd[:], std[:])
                # keep the PE HAM-warm across this serial chain: dummy f32
                # matmuls data-chained on each stat tile (results unread)
                for wi, wsrc in enumerate((mu, msq, var, std, rstd)):
                    dmy = ps_stat.tile([1, 512], F32, tag="ps_mu",
                                       name=f"dmy_{wi}")[:, :min(512, sp)]
                    nc.tensor.matmul(dmy, ones_f[0:1, 0:1],
                                     wsrc[:, 0:min(512, sp)],
                                     start=True, stop=True)

                Rmu = bc_pool.tile([P, sp], F32, tag="Rmu")
                nc.gpsimd.partition_broadcast(Rmu[:], mu[:])
                Rrstd = bc_pool.tile([P, sp], F32, tag="Rrstd")
                nc.gpsimd.partition_broadcast(Rrstd[:], rstd[:])

                for dt in range(NT):
                    xf_t = stream.tile([P, sp], F32, tag="xf")
                    nc.sync.dma_start(xf_t[:], xf_d[dt])
                    xc = stream.tile([P, sp], F32, tag="xc")
                    nc.vector.tensor_sub(xc[:], xf_t[:], Rmu[:])
                    nc.vector.tensor_mul(xT[dt][:], xc[:], Rrstd[:])

                # v = x~ @ Wv^T  (v in [s, e] layout)
                wv = []
                for dt in range(NT):
                    t = wv_pool.tile([P, D], F32R, tag="wv")
                    nc.scalar.dma_start(t[:], wv_d[dt])
                    wv.append(t)
                for st in range(nts):
                    for (c0, cw) in ech:
                        ps = ps_mm.tile([P, 512], F32, tag="ps_mm", name="ps_v")[:, :cw]
                        for dt in range(NT):
                            nc.tensor.matmul(ps, xT[dt][:, st * P:(st + 1) * P],
                                             wv[dt][:, c0:c0 + cw],
                                             start=(dt == 0), stop=(dt == NT - 1))
                        nc.scalar.copy(v_sb[st][:, c0:c0 + cw], ps)

            # ---------- heads ----------
            with ExitStack() as sH:
                wqk_pool = sH.enter_context(tc.tile_pool(name="wqk", bufs=3))
                qk_pool = sH.enter_context(tc.tile_pool(name="qk", bufs=3))
                E_pool = sH.enter_context(
                    tc.tile_pool(name="E", bufs=max(8, 4 * nts - 4)))
                ysb_pool = sH.enter_context(tc.tile_pool(name="ysb", bufs=2))
                x2h_pool = sH.enter_context(tc.tile_pool(name="x2h", bufs=2))
                rz_pool = sH.enter_context(tc.tile_pool(name="rz", bufs=1))
                rzb_pool = sH.enter_context(tc.tile_pool(name="rzb", bufs=2))
                xf2_pool = sH.enter_context(tc.tile_pool(name="xf2", bufs=2))
                ps_s_pool = sH.enter_context(
                    tc.tile_pool(name="ps_s", bufs=2, space="PSUM"))
                ps_z_pool = sH.enter_context(
                    tc.tile_pool(name="ps_z", bufs=2, space="PSUM"))
                ps_y_pool = sH.enter_context(
                    tc.tile_pool(name="ps_y", bufs=2, space="PSUM"))

                wqk_tiles = {}

                def _load_wqk(hh):
                    t = wqk_pool.tile([P, NT, 2 * P], F32R, tag="wqk",
                                      name=f"wqk_{hh}")
                    nc.sync.dma_start(t[:], wqk_d[hh].rearrange("t p e -> p t e"))
                    wqk_tiles[hh] = t

                _load_wqk(0)
                if H > 1:
                    _load_wqk(1)
                for h in range(H):
                    if h + 2 < H:
                        _load_wqk(h + 2)
                    xf2 = xf2_pool.tile([P, sp], F32, tag="xf2")
                    nc.sync.dma_start(xf2[:], xf_d[h])
                    wqk_h = wqk_tiles.pop(h)

                    q_sb = qk_pool.tile([P, sp], F32R, tag="qk")
                    k_sb = qk_pool.tile([P, sp], F32R, tag="qk")
                    for (c0, cw) in sch:
                        ps = ps_mm.tile([P, 512], F32, tag="ps_mm", name="ps_q")[:, :cw]
                        for dt in range(NT):
                            nc.tensor.matmul(ps, wqk_h[:, dt, 0:P],
                                             xT[dt][:, c0:c0 + cw],
                                             start=(dt == 0), stop=(dt == NT - 1))
                        nc.scalar.copy(q_sb[:, c0:c0 + cw], ps)
                        ps2 = ps_mm.tile([P, 512], F32, tag="ps_mm", name="ps_k")[:, :cw]
                        for dt in range(NT):
                            nc.tensor.matmul(ps2, wqk_h[:, dt, P:2 * P],
                                             xT[dt][:, c0:c0 + cw],
                                             start=(dt == 0), stop=(dt == NT - 1))
                        nc.scalar.copy(k_sb[:, c0:c0 + cw], ps2)

                    ps_z = [ps_z_pool.tile([1, cw], F32, tag="ps_z", name=f"ps_z_{i}")
                            for i, (_, cw) in enumerate(sch)]
                    ps_y = [ps_y_pool.tile([P, cw], F32, tag="ps_y", name=f"ps_y_{i}")
                            for i, (_, cw) in enumerate(sch)]
                    E = {}
                    for kt in range(nts):
                        for ci, (c0, cw) in enumerate(sch):
                            Et = E_pool.tile([P, 512], F32R, tag="E", name="Et")[:, :cw]
                            ps = ps_s_pool.tile([P, 512], F32, tag="ps_s", name="ps_sc")[:, :cw]
                            nc.tensor.matmul(ps, k_sb[:, kt * P:(kt + 1) * P],
                                             q_sb[:, c0:c0 + cw],
                                             start=True, stop=True)
                            nc.scalar.activation(Et, ps, AF.Exp,
                                                 bias=maskT[:, kt:kt + 1])
                            nc.tensor.matmul(ps_z[ci][:], ones[:], Et,
                                             start=(kt == 0), stop=(kt == nts - 1))
                            nc.tensor.matmul(ps_y[ci][:],
                                             v_sb[kt][:, h * P:(h + 1) * P], Et,
                                             start=(kt == 0), stop=(kt == nts - 1))
                            E[(kt, ci)] = Et

                    z = rz_pool.tile([1, sp], F32, tag="z")
                    for ci, (c0, cw) in enumerate(sch):
                        nc.scalar.copy(z[:, c0:c0 + cw], ps_z[ci][:])
                    rz = rz_pool.tile([1, sp], F32, tag="rz")
                    nc.vector.reciprocal_approx_fast(rz[:], z[:])
                    Rz = rzb_pool.tile([P, sp], F32, tag="Rz")
                    nc.gpsimd.partition_broadcast(Rz[:], rz[:])

                    # free ps_y early: stage y into SBUF via ACT
                    y_sb = ysb_pool.tile([P, sp], F32, tag="y_sb")
                    for ci, (c0, cw) in enumerate(sch):
                        nc.scalar.copy(y_sb[:, c0:c0 + cw], ps_y[ci][:])

                    # x2^T[h] = y^T_h / Z + x[h]  (before the att flood so its
                    # store lands early on the sync queue)
                    x2h = x2h_pool.tile([P, sp], F32, tag="x2h")
                    for ci, (c0, cw) in enumerate(sch):
                        nc.vector.tensor_mul(x2h[:, c0:c0 + cw],
                                             y_sb[:, c0:c0 + cw],
                                             Rz[:, c0:c0 + cw])
                        nc.vector.tensor_add(x2h[:, c0:c0 + cw],
                                             x2h[:, c0:c0 + cw],
                                             xf2[:, c0:c0 + cw])
                    nc.sync.dma_start(x2_dram[h], x2h[:])

                    # att rows: normalize E in place (DVE/GPSIMD split) + stream
                    for kt in range(nts):
                        for ci, (c0, cw) in enumerate(sch):
                            Et = E[(kt, ci)]
                            eng = (nc.gpsimd if (h == H - 1 or kt % 3 == 2)
                                   else nc.vector)
                            eng.tensor_mul(Et, Et.bitcast(F32), Rz[:, c0:c0 + cw])
                            nc.sync.dma_start(
                                att_d[h, kt * P:(kt + 1) * P, c0:c0 + cw], Et)

        # ---------- phase 3: LN2 + FC + proj ----------
        with ExitStack() as sB:
            x2f_pool = sB.enter_context(tc.tile_pool(name="x2f", bufs=NT))
            sq2_pool = sB.enter_context(tc.tile_pool(name="sq2", bufs=2))
            stat2_pool = sB.enter_context(tc.tile_pool(name="stat2", bufs=1))
            bc2_pool = sB.enter_context(tc.tile_pool(name="bc2", bufs=1))
            xt2_pool = sB.enter_context(tc.tile_pool(name="xt2", bufs=NT))
            w2_pool = sB.enter_context(tc.tile_pool(name="w2", bufs=NT + 1))
            hT_pool = sB.enter_context(tc.tile_pool(name="hT", bufs=NT))
            tmp_pool = sB.enter_context(tc.tile_pool(name="tmp", bufs=3))
            ps_mm2 = sB.enter_context(
                tc.tile_pool(name="ps_mm2", bufs=4, space="PSUM"))
            ps_stat2 = sB.enter_context(
                tc.tile_pool(name="ps_stat2", bufs=2, space="PSUM"))

            ps_mu2 = [ps_stat2.tile([1, cw], F32, tag="ps_mu2", name=f"ps_mu2_{i}")
                      for i, (_, cw) in enumerate(sch)]
            ps_sq2 = [ps_stat2.tile([1, cw], F32, tag="ps_sq2", name=f"ps_sq2_{i}")
                      for i, (_, cw) in enumerate(sch)]
            x2fs = []
            for dt in range(NT):
                x2f = x2f_pool.tile([P, sp], F32, tag="x2f", name=f"x2f_{dt}")
                nc.scalar.dma_start(x2f[:], x2_dram[dt])
                x2fs.append(x2f)
                r = sq2_pool.tile([P, sp], F32R, tag="x2r")
                nc.scalar.copy(r[:], x2f[:])
                q = sq2_pool.tile([P, sp], F32R, tag="x2sq")
                nc.scalar.activation(q[:], x2f[:], AF.Square)
                for ci, (c0, cw) in enumerate(sch):
                    nc.tensor.matmul(ps_mu2[ci][:], ones[:], r[:, c0:c0 + cw],
                                     start=(dt == 0), stop=(dt == NT - 1))
                for ci, (c0, cw) in enumerate(sch):
                    nc.tensor.matmul(ps_sq2[ci][:], ones[:], q[:, c0:c0 + cw],
                                     start=(dt == 0), stop=(dt == NT - 1))

            mu2 = stat2_pool.tile([1, sp], F32, tag="mu2")
            msq2 = stat2_pool.tile([1, sp], F32, tag="msq2")
            for ci, (c0, cw) in enumerate(sch):
                nc.scalar.mul(mu2[:, c0:c0 + cw], ps_mu2[ci][:], 1.0 / D)
                nc.scalar.mul(msq2[:, c0:c0 + cw], ps_sq2[ci][:], 1.0 / D)
            var2 = stat2_pool.tile([1, sp], F32, tag="var2")
            nc.vector.tensor_mul(var2[:], mu2[:], mu2[:])
            nc.vector.tensor_sub(var2[:], msq2[:], var2[:])
            std2 = stat2_pool.tile([1, sp], F32, tag="std2")
            nc.scalar.activation(std2[:], var2[:], AF.Sqrt, bias=eps_c[:])
            rstd2 = stat2_pool.tile([1, sp], F32, tag="rstd2")
            nc.vector.reciprocal_approx_fast(rstd2[:], std2[:])
            for wi, wsrc in enumerate((mu2, msq2, var2, std2, rstd2)):
                dmy2 = ps_stat2.tile([1, 512], F32, tag="ps_mu2",
                                     name=f"dmy2_{wi}")[:, :min(512, sp)]
                nc.tensor.matmul(dmy2, ones_f[0:1, 0:1],
                                 wsrc[:, 0:min(512, sp)],
                                 start=True, stop=True)
            Rmu2 = bc2_pool.tile([P, sp], F32, tag="Rmu2")
            nc.gpsimd.partition_broadcast(Rmu2[:], mu2[:])
            Rrstd2 = bc2_pool.tile([P, sp], F32, tag="Rrstd2")
            nc.gpsimd.partition_broadcast(Rrstd2[:], rstd2[:])

            xt2 = []
            for dt in range(NT):
                t = tmp_pool.tile([P, sp], F32, tag="x2c")
                nc.vector.tensor_sub(t[:], x2fs[dt][:], Rmu2[:])
                u = xt2_pool.tile([P, sp], F32R, tag="xt2")
                nc.vector.tensor_mul(u[:], t[:], Rrstd2[:])
                xt2.append(u)

            # FC + exact gelu: g = h * (0.5 * erf(h/sqrt(2)) + 0.5)
            wfc = []
            for dt in range(NT):
                t = w2_pool.tile([P, D], F32R, tag="w2")
                nc.scalar.dma_start(t[:], wfc_d[dt])
                wfc.append(t)
            hT = [hT_pool.tile([P, sp], F32R, tag="hT", name=f"hT_{i}") for i in range(NT)]
            for et in range(NT):
                for (c0, cw) in sch:
                    ps = ps_mm2.tile([P, 512], F32, tag="ps_mm2", name="ps_fc")[:, :cw]
                    for dt in range(NT):
                        nc.tensor.matmul(ps, wfc[dt][:, et * P:(et + 1) * P],
                                         xt2[dt][:, c0:c0 + cw],
                                         start=(dt == 0), stop=(dt == NT - 1))
                    nc.scalar.activation(hT[et][:, c0:c0 + cw], ps, AF.Gelu)

            wpj = []
            for et in range(NT):
                t = w2_pool.tile([P, D], F32R, tag="w2")
                nc.scalar.dma_start(t[:], wpj_d[et])
                wpj.append(t)
            for dt in range(NT):
                for (c0, cw) in sch:
                    ps = ps_mm2.tile([P, 512], F32, tag="ps_mm2", name="ps_pj")[:, :cw]
                    for et in range(NT):
                        nc.tensor.matmul(ps, wpj[et][:, dt * P:(dt + 1) * P],
                                         hT[et][:, c0:c0 + cw],
                                         start=(et == 0), stop=(et == NT - 1))
                    o = tmp_pool.tile([P, 512], F32, tag="o", name="o")[:, :cw]
                    nc.vector.tensor_add(o, ps, xt2[dt][:, c0:c0 + cw].bitcast(F32))
                    nc.sync.dma_start(out_d[dt, :, c0:c0 + cw], o)

    nc.compile()
    return nc


def prep_core_inputs(x_b, size_b, ln1_w, qkv_w, ln2_w, fc_w, proj_w, sp=S):
    """Host-side prep for one core/sample."""
    nts = sp // P
    xT = np.ascontiguousarray(x_b[:sp].T)                     # [D, sp]
    wq = (qkv_w[0:D] * ln1_w[None, :]) / np.sqrt(HD)          # [e, d]
    wk = qkv_w[D:2 * D] * ln1_w[None, :]
    wv = qkv_w[2 * D:3 * D] * ln1_w[None, :]
    wqkT = np.concatenate([wq, wk], axis=0).T                 # [d, 2D]
    # [H, NT, P, 2P]: head h -> cols [h*128:(h+1)*128] of q and of k
    wqk = np.empty((H, NT, P, 2 * P), np.float32)
    for h in range(H):
        wqk[h, :, :, 0:P] = wqkT[:, h * P:(h + 1) * P].reshape(NT, P, P)
        wqk[h, :, :, P:2 * P] = wqkT[:, D + h * P:D + (h + 1) * P].reshape(NT, P, P)
    wvT = np.ascontiguousarray(wv.T).reshape(NT, P, D)
    wfcT = np.ascontiguousarray((fc_w * ln2_w[None, :]).T).reshape(NT, P, D)
    wpjT = np.ascontiguousarray(proj_w.T).reshape(NT, P, D)

    kp = np.arange(P)[:, None] + P * np.arange(nts)[None, :]  # [P, nts]
    maskT = np.where(kp < int(size_b), 0.0, NEG).astype(np.float32)

    return {
        "xr": round_f32r(xT).reshape(NT, P, sp),
        "xf": np.ascontiguousarray(xT.reshape(NT, P, sp), dtype=np.float32),
        "wqk": round_f32r(wqk),
        "wv": round_f32r(wvT),
        "wfc": round_f32r(wfcT),
        "wpj": round_f32r(wpjT),
        "maskT": maskT,
    }


_NC_CACHE = {}


def _get_nc(sp=S):
    if sp not in _NC_CACHE:
        _NC_CACHE[sp] = build(sp)
    return _NC_CACHE[sp]


def _install_ntff_hook():
    """Shim antenv.axon_hooks so trace=True works under axon (for test.py)."""
    import antenv
    if "antenv.axon_hooks" in sys.modules:
        return
    hooks_mod = types.ModuleType("antenv.axon_hooks")
    _hook = [None]
    hooks_mod.set_axon_ntff_profile_hook = lambda h: _hook.__setitem__(0, h)
    hooks_mod.get_axon_ntff_profile_hook = lambda: _hook[0]
    sys.modules["antenv.axon_hooks"] = hooks_mod
    antenv.axon_hooks = hooks_mod
    try:
        if "/root/.axon_site" not in sys.path:
            sys.path.insert(0, "/root/.axon_site")
        from trn_agent_boot.trn_boot import _ntff_profile_via_ctypes
        h = _ntff_profile_via_ctypes("/opt/axon/libaxon_pjrt.so")
        if h is not None:
            hooks_mod.set_axon_ntff_profile_hook(h)
    except Exception:
        pass


def run(inputs, sp=S, trace=False, **kwargs):
    """Shard, run on 8 cores, gather. Returns (out [B,sp,D], att [B,H,sp,sp], res)."""
    x = np.asarray(inputs["x"], np.float32)
    sizes = np.asarray(inputs["belief_base_sizes"])
    ln1_w = np.asarray(inputs["ln1_w"], np.float32)
    qkv_w = np.asarray(inputs["qkv_w"], np.float32)
    ln2_w = np.asarray(inputs["ln2_w"], np.float32)
    fc_w = np.asarray(inputs["fc_w"], np.float32)
    proj_w = np.asarray(inputs["proj_w"], np.float32)

    nc = _get_nc(sp)
    in_maps = [
        prep_core_inputs(x[b], min(int(sizes[b]), sp), ln1_w, qkv_w, ln2_w,
                         fc_w, proj_w, sp)
        for b in range(B)
    ]
    if trace:
        _install_ntff_hook()
    res = run_bass_kernel_spmd(nc, in_maps, core_ids=list(range(B)),
                               trace=trace, **kwargs)

    out = np.empty((B, sp, D), np.float32)
    att = np.empty((B, H, sp, sp), np.float32)
    for b in range(B):
        r = res.results[b]
        out[b] = r["outT"].reshape(D, sp).T
        att[b] = np.swapaxes(r["attT"], 1, 2)
    return out, att, res


def kernel(**inputs):
    out, att, _ = run(inputs, sp=S, trace=False)
    return out, att
